# revision 13
# baseline (speedup 1.0000x reference)
"""Luong attention on 8 Trainium2 NeuronCores (Bass/Tile).

Reference computation (per batch b):
    enc_proj = enc @ W_enc                      (S,E)@(E,D) -> (S,D)
    scores   = dec @ enc_proj^T                 (T,D)@(D,S) -> (T,S)
    wts      = softmax(mask(scores), axis=S)
    context  = wts @ enc                        (T,S)@(S,E) -> (T,E)
    out      = tanh([context|dec] @ W_fin)      (T,E+D)@(E+D,D) -> (T,D)

Kernel algebra (all matmuls contract on the partition dim, out = lhsT.T @ rhs):
    dec_projT = W_encT.T @ decT                 [e,t]   = dec @ W_enc^T, transposed
    scoresT   = encT.T @ dec_projT              [s,t]   (per 128-row s-tile)
    exp_sT    = exp(scoresT + bias)             bias = mask? -C : -1e30  (fused shift+mask)
    ctx_u|den = exp_sT.T @ [enc|1]              [t,257] (ones col -> softmax denominator)
    ctx       = ctx_u * recip(den)              per-partition scalar broadcast
    out       = tanh(catT.T @ W_fin)            catT = [ctxT; decT]

Sharding: data-parallel over batch, 2 batches/core ("slots" A and B). Batches are
paired long+short by valid s-tile count; the program statically runs K_A tiles for
slot A and K_B for slot B (max over cores), with fully-masked padding tiles zeroed
on the host. The softmax shift C is exact algebra (not an approximation).
"""

import math
import os
import sys

import numpy as np

for p in ("/opt/trn_rl_repo", "/root/.axon_site", "/root/.axon_site/_ro/trn_rl_repo",
          "/root/.axon_site/_ro/pypackages"):
    if os.path.isdir(p) and p not in sys.path:
        sys.path.append(p)

import ml_dtypes  # noqa: E402

import concourse.bass as bass  # noqa: E402
import concourse.tile as tile  # noqa: E402
from concourse import bacc, mybir  # noqa: E402
from concourse.bass_utils import run_bass_kernel_spmd  # noqa: E402
from concourse.masks import make_identity  # noqa: E402

P = 128
E = 256  # enc feature dim
D = 256  # dec feature dim
F = E + D
NCORES = 8
C_SHIFT = 35.0  # exp(score - C): keeps exp in fp32 range for |score| <~ 115
MASK_BIAS = -1e30

bf16 = ml_dtypes.bfloat16

# scores matmul dtype: "f32r" (1 cyc/col, HW-reduced rounding), "f16" (1 cyc/col,
# 11-bit mantissa), or "f32" (4 cyc/col, exact)
SCORES_DT = os.environ.get("LUONG_SCORES_DT", "f32r")
CTX_DT = os.environ.get("LUONG_CTX_DT", "bf16")  # "bf16" or "f32r"
FIN_DT = os.environ.get("LUONG_FIN_DT", "f16")  # "f16" or "f32r"


def build_program(K_A, K_B, T, scores_dt=SCORES_DT, ctx_dt=CTX_DT, fin_dt=FIN_DT):
    """Build the 8-core SPMD Bass program. Returns (nc, meta)."""
    dt = mybir.dt
    KT = K_A + K_B
    NT = T // P  # t-tiles per batch

    nc = bacc.Bacc("TRN2", target_bir_lowering=False, debug=False,
                   num_devices=NCORES)

    f32, f32r, f16, dbf = dt.float32, dt.float32r, dt.float16, dt.bfloat16

    # float32r is a distinct reduced-rounding fp32 matmul format: walrus
    # requires every fp32r-matmul operand to be *declared/produced* as f32r
    # (DMA of an f32r DRAM tensor, or a DVE copy with f32r output).
    if scores_dt == "f16":
        enc_t_dt, enc_t_np = f16, np.float16
    elif scores_dt == "f32r":
        enc_t_dt, enc_t_np = f32r, np.float32
    else:
        enc_t_dt, enc_t_np = f32, np.float32
    dec_mm_dt = {"f32": f32, "f16": f16}.get(scores_dt, f32r)
    ctx_mm_dt = dbf if ctx_dt == "bf16" else f32r
    ctx_np = bf16 if ctx_dt == "bf16" else np.float32
    fin_mm_dt = f16 if fin_dt == "f16" else f32r
    fin_np = np.float16 if fin_dt == "f16" else np.float32

    # ---- DRAM I/O ----
    encT_d = nc.dram_tensor("encT_pack", [E, KT * P], enc_t_dt, kind="ExternalInput").ap()
    enc1_d = nc.dram_tensor("enc1_pack", [KT * P, E + 1], ctx_mm_dt, kind="ExternalInput").ap()
    bias_d = nc.dram_tensor("bias_pack", [P, KT], f32, kind="ExternalInput").ap()
    decT_d = nc.dram_tensor("decT", [2, D, T], dec_mm_dt, kind="ExternalInput").ap()
    decTf_d = nc.dram_tensor("decT_fin", [2, D, T], fin_mm_dt, kind="ExternalInput").ap()
    wencT_d = nc.dram_tensor("W_encT", [D, E], dec_mm_dt, kind="ExternalInput").ap()
    wfin_d = nc.dram_tensor("W_fin_c", [F, D], fin_mm_dt, kind="ExternalInput").ap()
    out_d = nc.dram_tensor("out", [2, T, D], f32, kind="ExternalOutput").ap()

    Exp = mybir.ActivationFunctionType.Exp
    Tanh = mybir.ActivationFunctionType.Tanh

    with tile.TileContext(nc) as tc:
        from contextlib import ExitStack
        with ExitStack() as ctx:
            const = ctx.enter_context(tc.tile_pool(name="const", bufs=1))
            encT_p = ctx.enter_context(tc.tile_pool(name="encT", bufs=2 * KT))
            enc1_p = ctx.enter_context(tc.tile_pool(name="enc1", bufs=KT))
            exp_p = ctx.enter_context(tc.tile_pool(name="exp", bufs=KT))
            dect_p = ctx.enter_context(tc.tile_pool(name="dect", bufs=4))
            dpt_p = ctx.enter_context(tc.tile_pool(name="dpt", bufs=4))
            cat_p = ctx.enter_context(tc.tile_pool(name="cat", bufs=8))
            ctx16_p = ctx.enter_context(tc.tile_pool(name="ctx16", bufs=4))
            recip_p = ctx.enter_context(tc.tile_pool(name="recip", bufs=4))
            out_p = ctx.enter_context(tc.tile_pool(name="outp", bufs=6))
            ps_big = ctx.enter_context(tc.tile_pool(name="ps_big", bufs=2, space="PSUM"))
            ps_sm = ctx.enter_context(tc.tile_pool(name="ps_sm", bufs=4, space="PSUM"))

            # ---- constants ----
            wencT = []
            for k in range(2):
                t = const.tile([P, E], dec_mm_dt, tag=f"wencT{k}")
                nc.sync.dma_start(out=t[:], in_=wencT_d[k * P:(k + 1) * P, :])
                wencT.append(t)
            wfin = []
            for k in range(4):
                t = const.tile([P, D], fin_mm_dt, tag=f"wfin{k}")
                nc.sync.dma_start(out=t[:], in_=wfin_d[k * P:(k + 1) * P, :])
                wfin.append(t)
            ident = const.tile([P, P], f16 if fin_dt == "f16" else dbf, tag="ident")
            make_identity(nc, ident[:])
            bias_t = const.tile([P, KT], f32, tag="bias")
            nc.sync.dma_start(out=bias_t[:], in_=bias_d[:, :])

            # ---- packed enc loads (per virtual s-tile for fine-grained deps) ----
            encT_tiles = []  # [e-chunk][tile]
            for c in range(2):
                row = []
                for j in range(KT):
                    t = encT_p.tile([P, P], enc_t_dt, tag="encT")
                    nc.sync.dma_start(
                        out=t[:],
                        in_=encT_d[c * P:(c + 1) * P, j * P:(j + 1) * P])
                    row.append(t)
                encT_tiles.append(row)
            enc1_tiles = []
            for j in range(KT):
                t = enc1_p.tile([P, E + 1], ctx_mm_dt, tag="enc1")
                nc.sync.dma_start(out=t[:], in_=enc1_d[j * P:(j + 1) * P, :])
                enc1_tiles.append(t)

            for slot, K_slot, j_base in ((0, K_A, 0), (1, K_B, K_A)):
                # ---- load decT (f32 for projection; fin dtype for catT rows) ----
                dect = []
                for c in range(2):
                    t = dect_p.tile([P, T], dec_mm_dt, tag="dect")
                    nc.sync.dma_start(out=t[:], in_=decT_d[slot, c * P:(c + 1) * P, :])
                    dect.append(t)
                catT = []
                for c in range(4):
                    t = cat_p.tile([P, T], fin_mm_dt, tag="cat")
                    catT.append(t)
                for c in range(2):
                    nc.sync.dma_start(out=catT[2 + c][:],
                                      in_=decTf_d[slot, c * P:(c + 1) * P, :])

                # ---- dec_projT[e,t] = W_encT.T @ decT  (f32r) ----
                dpt = []
                for m in range(2):
                    ps = ps_big.tile([P, T], f32, tag="big")
                    for h0 in range(0, T, 512):
                        hs = slice(h0, min(h0 + 512, T))
                        for k in range(2):
                            nc.tensor.matmul(
                                ps[:, hs],
                                lhsT=wencT[k][:, m * P:(m + 1) * P],
                                rhs=dect[k][:, hs],
                                start=(k == 0), stop=(k == 1))
                    t = dpt_p.tile([P, T], enc_t_dt, tag="dpt")
                    nc.vector.tensor_copy(out=t[:], in_=ps[:])
                    dpt.append(t)

                # ---- scoresT + exp per s-tile ----
                exp_tiles = []
                for j in range(K_slot):
                    jj = j_base + j
                    ps = ps_big.tile([P, T], f32, tag="big")
                    for h0 in range(0, T, 512):
                        hs = slice(h0, min(h0 + 512, T))
                        for k in range(2):
                            nc.tensor.matmul(
                                ps[:, hs],
                                lhsT=encT_tiles[k][jj][:],
                                rhs=dpt[k][:, hs],
                                start=(k == 0), stop=(k == 1))
                    ex = exp_p.tile([P, T], ctx_mm_dt, tag="exp")
                    nc.scalar.activation(ex[:], ps[:], Exp,
                                         bias=bias_t[:, jj:jj + 1], scale=1.0)
                    exp_tiles.append(ex)

                # ---- context accumulation + normalize + transpose ----
                for tm in range(NT):
                    tsl = slice(tm * P, (tm + 1) * P)
                    psc = ps_sm.tile([P, E + 1], f32, tag="sm")
                    for j in range(K_slot):
                        jj = j_base + j
                        nc.tensor.matmul(
                            psc[:],
                            lhsT=exp_tiles[j][:, tsl],
                            rhs=enc1_tiles[jj][:],
                            start=(j == 0), stop=(j == K_slot - 1))
                    rc = recip_p.tile([P, 1], f32, tag="recip")
                    nc.vector.reciprocal(rc[:], psc[:, E:E + 1])
                    cx = ctx16_p.tile([P, E], fin_mm_dt, tag="ctx16")
                    nc.vector.tensor_scalar_mul(cx[:], psc[:, 0:E], rc[:])
                    for h in range(2):
                        pst = ps_sm.tile([P, P], fin_mm_dt, tag="sm")
                        nc.tensor.transpose(pst[:], cx[:, h * P:(h + 1) * P], ident[:])
                        nc.vector.tensor_copy(out=catT[h][:, tsl], in_=pst[:])

                # ---- final matmul + tanh + store ----
                for tm in range(NT):
                    tsl = slice(tm * P, (tm + 1) * P)
                    psf = ps_sm.tile([P, D], f32, tag="sm")
                    for k in range(4):
                        nc.tensor.matmul(
                            psf[:],
                            lhsT=catT[k][:, tsl],
                            rhs=wfin[k][:],
                            start=(k == 0), stop=(k == 3))
                    ot = out_p.tile([P, D], f32, tag="outp")
                    nc.scalar.activation(ot[:], psf[:], Tanh)
                    nc.sync.dma_start(out=out_d[slot, tsl, :], in_=ot[:])

    nc.compile()
    meta = dict(enc_t_np=enc_t_np, ctx_np=ctx_np, fin_np=fin_np, KT=KT,
                dec_np=np.float16 if scores_dt == "f16" else np.float32)
    return nc, meta


def pack_inputs(inputs, K_A, K_B, A_batches, B_batches, ktiles, meta, T):
    """Build the per-core in_maps."""
    enc = np.ascontiguousarray(inputs["enc_states"])
    dec = np.ascontiguousarray(inputs["dec_states"])
    mask = np.ascontiguousarray(inputs["src_mask"])
    W_enc = np.ascontiguousarray(inputs["W_enc"])
    W_fin = np.ascontiguousarray(inputs["W_fin"])
    B, S, _ = enc.shape
    KT = K_A + K_B
    enc_t_np, ctx_np, fin_np = meta["enc_t_np"], meta["ctx_np"], meta["fin_np"]

    wencT = np.ascontiguousarray(W_enc.T).astype(meta["dec_np"])
    wfin_c = W_fin.astype(fin_np)

    in_maps = []
    for c in range(NCORES):
        pair = (A_batches[c], B_batches[c])
        encT_pack = np.zeros((E, KT * P), enc_t_np)
        enc1_pack = np.zeros((KT * P, E + 1), ctx_np)
        bias_pack = np.full((P, KT), MASK_BIAS, np.float32)
        for slot, b in enumerate(pair):
            jb = 0 if slot == 0 else K_A
            kt = ktiles[b]
            n = min(S, kt * P)
            encT_pack[:, jb * P:jb * P + n] = enc[b, :n, :].T.astype(enc_t_np)
            enc1_pack[jb * P:jb * P + n, :E] = enc[b, :n, :].astype(ctx_np)
            enc1_pack[jb * P:jb * P + kt * P, E] = ctx_np(1.0)
            mrow = np.where(mask[b, :n], np.float32(-C_SHIFT),
                            np.float32(MASK_BIAS))
            bp = np.full((kt * P,), MASK_BIAS, np.float32)
            bp[:n] = mrow
            bias_pack[:, jb:jb + kt] = bp.reshape(kt, P).T
        decT = dec[list(pair)].transpose(0, 2, 1).astype(np.float32)
        decT = np.ascontiguousarray(decT)
        in_maps.append({
            "encT_pack": encT_pack,
            "enc1_pack": enc1_pack,
            "bias_pack": bias_pack,
            "decT": decT.astype(meta["dec_np"]),
            "decT_fin": decT.astype(fin_np),
            "W_encT": wencT,
            "W_fin_c": wfin_c,
        })
    return in_maps


_PROGRAM_CACHE = {}


def _plan(src_mask, S, T):
    lengths = src_mask.sum(1).astype(np.int64)
    ktiles = np.maximum(1, (lengths + P - 1) // P).astype(np.int64)
    order = np.argsort(-ktiles, kind="stable")
    A_batches = [int(x) for x in order[:NCORES]]
    B_batches = [int(x) for x in order[NCORES:][::-1]]
    K_A = int(ktiles[A_batches].max())
    K_B = int(ktiles[B_batches].max())
    return A_batches, B_batches, ktiles, K_A, K_B


def kernel(enc_states, dec_states, src_mask, W_enc, W_fin, _trace=False):
    inputs = dict(enc_states=enc_states, dec_states=dec_states,
                  src_mask=src_mask, W_enc=W_enc, W_fin=W_fin)
    B, S, _ = enc_states.shape
    T = dec_states.shape[1]
    A_batches, B_batches, ktiles, K_A, K_B = _plan(np.asarray(src_mask), S, T)

    key = (K_A, K_B, T, SCORES_DT, CTX_DT, FIN_DT)
    if key not in _PROGRAM_CACHE:
        _PROGRAM_CACHE[key] = build_program(K_A, K_B, T)
    nc, meta = _PROGRAM_CACHE[key]

    in_maps = pack_inputs(inputs, K_A, K_B, A_batches, B_batches, ktiles, meta, T)
    res = run_bass_kernel_spmd(nc, in_maps, core_ids=list(range(NCORES)),
                               trace=_trace)

    out = np.zeros((B, T, D), np.float32)
    for c in range(NCORES):
        o = res.results[c]["out"]
        out[A_batches[c]] = o[0]
        out[B_batches[c]] = o[1]
    if _trace:
        kernel._last_results = res
    return out


# revision 15
# speedup vs baseline: 1.2975x; 1.2975x over previous
"""Luong attention on 8 Trainium2 NeuronCores (Bass/Tile).

Reference computation (per batch b):
    enc_proj = enc @ W_enc                      (S,E)@(E,D) -> (S,D)
    scores   = dec @ enc_proj^T                 (T,D)@(D,S) -> (T,S)
    wts      = softmax(mask(scores), axis=S)
    context  = wts @ enc                        (T,S)@(S,E) -> (T,E)
    out      = tanh([context|dec] @ W_fin)      (T,E+D)@(E+D,D) -> (T,D)

Kernel algebra (all matmuls contract on the partition dim, out = lhsT.T @ rhs):
    dec_projT = W_encT.T @ decT                 [e,t]   = dec @ W_enc^T, transposed
    scoresT   = encT.T @ dec_projT              [s,t]   (per 128-row s-tile)
    exp_sT    = exp(scoresT + bias)             bias = mask? -C : -1e30  (fused shift+mask)
    ctx_u|den = exp_sT.T @ [enc|1]              [t,257] (ones col -> softmax denominator)
    ctx       = ctx_u * recip(den)              per-partition scalar broadcast
    out       = tanh(catT.T @ W_fin)            catT = [ctxT; decT]

Sharding: data-parallel over batch, 2 batches/core ("slots" A and B). Batches are
paired long+short by valid s-tile count; the program statically runs K_A tiles for
slot A and K_B for slot B (max over cores), with fully-masked padding tiles zeroed
on the host. The softmax shift C is exact algebra (not an approximation).
"""

import math
import os
import sys

import numpy as np

for p in ("/opt/trn_rl_repo", "/root/.axon_site", "/root/.axon_site/_ro/trn_rl_repo",
          "/root/.axon_site/_ro/pypackages"):
    if os.path.isdir(p) and p not in sys.path:
        sys.path.append(p)

import ml_dtypes  # noqa: E402

import concourse.bass as bass  # noqa: E402
import concourse.tile as tile  # noqa: E402
from concourse import bacc, mybir  # noqa: E402
from concourse.bass_utils import run_bass_kernel_spmd  # noqa: E402
from concourse.masks import make_identity  # noqa: E402

P = 128
E = 256  # enc feature dim
D = 256  # dec feature dim
F = E + D
NCORES = 8
C_SHIFT = 35.0  # exp(score - C): keeps exp in fp32 range for |score| <~ 115
MASK_BIAS = -1e30

bf16 = ml_dtypes.bfloat16

# scores matmul dtype: "f32r" (1 cyc/col, HW-reduced rounding), "f16" (1 cyc/col,
# 11-bit mantissa), or "f32" (4 cyc/col, exact)
SCORES_DT = os.environ.get("LUONG_SCORES_DT", "f32r")
CTX_DT = os.environ.get("LUONG_CTX_DT", "bf16")  # "bf16" or "f32r"
FIN_DT = os.environ.get("LUONG_FIN_DT", "f16")  # "f16" or "f32r"


def build_program(K_A, K_B, T, scores_dt=SCORES_DT, ctx_dt=CTX_DT, fin_dt=FIN_DT):
    """Build the 8-core SPMD Bass program. Returns (nc, meta)."""
    dt = mybir.dt
    KT = K_A + K_B
    NT = T // P  # t-tiles per batch

    nc = bacc.Bacc("TRN2", target_bir_lowering=False, debug=False,
                   num_devices=NCORES)

    f32, f32r, f16, dbf = dt.float32, dt.float32r, dt.float16, dt.bfloat16

    # float32r is a distinct reduced-rounding fp32 matmul format: walrus
    # requires every fp32r-matmul operand to be *declared/produced* as f32r
    # (DMA of an f32r DRAM tensor, or a DVE copy with f32r output).
    if scores_dt == "f16":
        enc_t_dt, enc_t_np = f16, np.float16
    elif scores_dt == "f32r":
        enc_t_dt, enc_t_np = f32r, np.float32
    else:
        enc_t_dt, enc_t_np = f32, np.float32
    dec_mm_dt = {"f32": f32, "f16": f16}.get(scores_dt, f32r)
    ctx_mm_dt = dbf if ctx_dt == "bf16" else f32r
    ctx_np = bf16 if ctx_dt == "bf16" else np.float32
    fin_mm_dt = f16 if fin_dt == "f16" else f32r
    fin_np = np.float16 if fin_dt == "f16" else np.float32

    # ---- DRAM I/O ----
    encT_d = nc.dram_tensor("encT_pack", [E, KT * P], enc_t_dt, kind="ExternalInput").ap()
    enc1_d = nc.dram_tensor("enc1_pack", [KT * P, E + 1], ctx_mm_dt, kind="ExternalInput").ap()
    bias_d = nc.dram_tensor("bias_pack", [P, KT], f32, kind="ExternalInput").ap()
    decT_d = nc.dram_tensor("decT", [2, D, T], dec_mm_dt, kind="ExternalInput").ap()
    wencT_d = nc.dram_tensor("W_encT", [D, E], dec_mm_dt, kind="ExternalInput").ap()
    wfin_d = nc.dram_tensor("W_fin_c", [F, D], fin_mm_dt, kind="ExternalInput").ap()
    out_d = nc.dram_tensor("out", [2, T, D], f32, kind="ExternalOutput").ap()

    Exp = mybir.ActivationFunctionType.Exp
    Tanh = mybir.ActivationFunctionType.Tanh

    with tile.TileContext(nc) as tc:
        from contextlib import ExitStack
        with ExitStack() as ctx:
            const = ctx.enter_context(tc.tile_pool(name="const", bufs=1))
            encT_p = ctx.enter_context(tc.tile_pool(name="encT", bufs=2))
            enc1_p = ctx.enter_context(tc.tile_pool(name="enc1", bufs=1))
            exp_p = ctx.enter_context(tc.tile_pool(name="exp", bufs=KT))
            dect_p = ctx.enter_context(tc.tile_pool(name="dect", bufs=4))
            dpt_p = ctx.enter_context(tc.tile_pool(name="dpt", bufs=4))
            cat_p = ctx.enter_context(tc.tile_pool(name="cat", bufs=8))
            ctx16_p = ctx.enter_context(tc.tile_pool(name="ctx16", bufs=4))
            recip_p = ctx.enter_context(tc.tile_pool(name="recip", bufs=4))
            out_p = ctx.enter_context(tc.tile_pool(name="outp", bufs=6))
            ps_big = ctx.enter_context(tc.tile_pool(name="ps_big", bufs=2, space="PSUM"))
            ps_sm = ctx.enter_context(tc.tile_pool(name="ps_sm", bufs=4, space="PSUM"))

            # ---- constants ----
            wencT = []
            for k in range(2):
                t = const.tile([P, E], dec_mm_dt, tag=f"wencT{k}")
                nc.sync.dma_start(out=t[:], in_=wencT_d[k * P:(k + 1) * P, :])
                wencT.append(t)
            wfin = []
            for k in range(4):
                t = const.tile([P, D], fin_mm_dt, tag=f"wfin{k}")
                nc.sync.dma_start(out=t[:], in_=wfin_d[k * P:(k + 1) * P, :])
                wfin.append(t)
            ident = const.tile([P, P], f16 if fin_dt == "f16" else dbf, tag="ident")
            make_identity(nc, ident[:])
            bias_t = const.tile([P, KT], f32, tag="bias")
            nc.sync.dma_start(out=bias_t[:], in_=bias_d[:, :])

            # ---- decT first on the sync ring: dp matmul needs it immediately ----
            dect_by_slot = []
            for slot in range(2):
                dect = []
                for c in range(2):
                    t = dect_p.tile([P, T], dec_mm_dt, tag="dect")
                    nc.sync.dma_start(out=t[:], in_=decT_d[slot, c * P:(c + 1) * P, :])
                    dect.append(t)
                dect_by_slot.append(dect)

            # ---- bulk enc loads: 3 big DMAs on the gpsimd (SWDGE) ring so they
            # don't serialize behind/ahead of the sync-ring loads ----
            encT_chunks = []
            for c in range(2):
                t = encT_p.tile([P, KT * P], enc_t_dt, tag="encT")
                nc.gpsimd.dma_start(out=t[:], in_=encT_d[c * P:(c + 1) * P, :])
                encT_chunks.append(t)
            enc1_big = enc1_p.tile([P, KT, E + 1], ctx_mm_dt, tag="enc1")
            nc.gpsimd.dma_start(out=enc1_big[:],
                                in_=enc1_d.rearrange("(j p) c -> p j c", p=P))

            assert dec_mm_dt == fin_mm_dt, (
                "catT reuses decT tiles directly; scores/fin dtype combo "
                f"{scores_dt}/{fin_dt} would need a converting copy")
            for slot, K_slot, j_base in ((0, K_A, 0), (1, K_B, K_A)):
                dect = dect_by_slot[slot]
                # dec is already in the final-matmul dtype: reuse as catT rows 2,3
                catT = []
                for c in range(2):
                    t = cat_p.tile([P, T], fin_mm_dt, tag="cat")
                    catT.append(t)
                catT += dect

                # ---- dec_projT[e,t] = W_encT.T @ decT  (f32r) ----
                dpt = []
                for m in range(2):
                    ps = ps_big.tile([P, T], f32, tag="big")
                    for h0 in range(0, T, 512):
                        hs = slice(h0, min(h0 + 512, T))
                        for k in range(2):
                            nc.tensor.matmul(
                                ps[:, hs],
                                lhsT=wencT[k][:, m * P:(m + 1) * P],
                                rhs=dect[k][:, hs],
                                start=(k == 0), stop=(k == 1))
                    t = dpt_p.tile([P, T], enc_t_dt, tag="dpt")
                    nc.vector.tensor_copy(out=t[:], in_=ps[:])
                    dpt.append(t)

                # ---- scoresT + exp per s-tile ----
                exp_tiles = []
                for j in range(K_slot):
                    jj = j_base + j
                    ps = ps_big.tile([P, T], f32, tag="big")
                    for h0 in range(0, T, 512):
                        hs = slice(h0, min(h0 + 512, T))
                        for k in range(2):
                            nc.tensor.matmul(
                                ps[:, hs],
                                lhsT=encT_chunks[k][:, jj * P:(jj + 1) * P],
                                rhs=dpt[k][:, hs],
                                start=(k == 0), stop=(k == 1))
                    ex = exp_p.tile([P, T], ctx_mm_dt, tag="exp")
                    nc.scalar.activation(ex[:], ps[:], Exp,
                                         bias=bias_t[:, jj:jj + 1], scale=1.0)
                    exp_tiles.append(ex)

                # ---- context accumulation + normalize + transpose ----
                for tm in range(NT):
                    tsl = slice(tm * P, (tm + 1) * P)
                    psc = ps_sm.tile([P, E + 1], f32, tag="sm")
                    for j in range(K_slot):
                        jj = j_base + j
                        nc.tensor.matmul(
                            psc[:],
                            lhsT=exp_tiles[j][:, tsl],
                            rhs=enc1_big[:, jj, :],
                            start=(j == 0), stop=(j == K_slot - 1))
                    rc = recip_p.tile([P, 1], f32, tag="recip")
                    nc.vector.reciprocal(rc[:], psc[:, E:E + 1])
                    cx = ctx16_p.tile([P, E], fin_mm_dt, tag="ctx16")
                    nc.vector.tensor_scalar_mul(cx[:], psc[:, 0:E], rc[:])
                    for h in range(2):
                        pst = ps_sm.tile([P, P], fin_mm_dt, tag="sm")
                        nc.tensor.transpose(pst[:], cx[:, h * P:(h + 1) * P], ident[:])
                        nc.vector.tensor_copy(out=catT[h][:, tsl], in_=pst[:])

                # ---- final matmul + tanh + store ----
                for tm in range(NT):
                    tsl = slice(tm * P, (tm + 1) * P)
                    psf = ps_sm.tile([P, D], f32, tag="sm")
                    for k in range(4):
                        nc.tensor.matmul(
                            psf[:],
                            lhsT=catT[k][:, tsl],
                            rhs=wfin[k][:],
                            start=(k == 0), stop=(k == 3))
                    ot = out_p.tile([P, D], f32, tag="outp")
                    nc.scalar.activation(ot[:], psf[:], Tanh)
                    nc.sync.dma_start(out=out_d[slot, tsl, :], in_=ot[:])

    nc.compile()
    meta = dict(enc_t_np=enc_t_np, ctx_np=ctx_np, fin_np=fin_np, KT=KT,
                dec_np=np.float16 if scores_dt == "f16" else np.float32)
    return nc, meta


def pack_inputs(inputs, K_A, K_B, A_batches, B_batches, ktiles, meta, T):
    """Build the per-core in_maps."""
    enc = np.ascontiguousarray(inputs["enc_states"])
    dec = np.ascontiguousarray(inputs["dec_states"])
    mask = np.ascontiguousarray(inputs["src_mask"])
    W_enc = np.ascontiguousarray(inputs["W_enc"])
    W_fin = np.ascontiguousarray(inputs["W_fin"])
    B, S, _ = enc.shape
    KT = K_A + K_B
    enc_t_np, ctx_np, fin_np = meta["enc_t_np"], meta["ctx_np"], meta["fin_np"]

    wencT = np.ascontiguousarray(W_enc.T).astype(meta["dec_np"])
    wfin_c = W_fin.astype(fin_np)

    in_maps = []
    for c in range(NCORES):
        pair = (A_batches[c], B_batches[c])
        encT_pack = np.zeros((E, KT * P), enc_t_np)
        enc1_pack = np.zeros((KT * P, E + 1), ctx_np)
        bias_pack = np.full((P, KT), MASK_BIAS, np.float32)
        for slot, b in enumerate(pair):
            jb = 0 if slot == 0 else K_A
            kt = ktiles[b]
            n = min(S, kt * P)
            encT_pack[:, jb * P:jb * P + n] = enc[b, :n, :].T.astype(enc_t_np)
            enc1_pack[jb * P:jb * P + n, :E] = enc[b, :n, :].astype(ctx_np)
            enc1_pack[jb * P:jb * P + kt * P, E] = ctx_np(1.0)
            mrow = np.where(mask[b, :n], np.float32(-C_SHIFT),
                            np.float32(MASK_BIAS))
            bp = np.full((kt * P,), MASK_BIAS, np.float32)
            bp[:n] = mrow
            bias_pack[:, jb:jb + kt] = bp.reshape(kt, P).T
        decT = dec[list(pair)].transpose(0, 2, 1).astype(np.float32)
        decT = np.ascontiguousarray(decT)
        in_maps.append({
            "encT_pack": encT_pack,
            "enc1_pack": enc1_pack,
            "bias_pack": bias_pack,
            "decT": decT.astype(meta["dec_np"]),
            "W_encT": wencT,
            "W_fin_c": wfin_c,
        })
    return in_maps


_PROGRAM_CACHE = {}


def _plan(src_mask, S, T):
    lengths = src_mask.sum(1).astype(np.int64)
    ktiles = np.maximum(1, (lengths + P - 1) // P).astype(np.int64)
    order = np.argsort(-ktiles, kind="stable")
    A_batches = [int(x) for x in order[:NCORES]]
    B_batches = [int(x) for x in order[NCORES:][::-1]]
    K_A = int(ktiles[A_batches].max())
    K_B = int(ktiles[B_batches].max())
    return A_batches, B_batches, ktiles, K_A, K_B


def kernel(enc_states, dec_states, src_mask, W_enc, W_fin, _trace=False):
    inputs = dict(enc_states=enc_states, dec_states=dec_states,
                  src_mask=src_mask, W_enc=W_enc, W_fin=W_fin)
    B, S, _ = enc_states.shape
    T = dec_states.shape[1]
    A_batches, B_batches, ktiles, K_A, K_B = _plan(np.asarray(src_mask), S, T)

    key = (K_A, K_B, T, SCORES_DT, CTX_DT, FIN_DT)
    if key not in _PROGRAM_CACHE:
        _PROGRAM_CACHE[key] = build_program(K_A, K_B, T)
    nc, meta = _PROGRAM_CACHE[key]

    in_maps = pack_inputs(inputs, K_A, K_B, A_batches, B_batches, ktiles, meta, T)
    res = run_bass_kernel_spmd(nc, in_maps, core_ids=list(range(NCORES)),
                               trace=_trace)

    out = np.zeros((B, T, D), np.float32)
    for c in range(NCORES):
        o = res.results[c]["out"]
        out[A_batches[c]] = o[0]
        out[B_batches[c]] = o[1]
    if _trace:
        kernel._last_results = res
    return out
